# revision 18
# baseline (speedup 1.0000x reference)
"""Trainium2 Bass kernel for nn_FeatureGenKerasV2.

Contract: kernel(x) with x [100000, 115, 3] f32 -> [1, 200, 1198] f32.

Reference semantics:
  - global: cond = (count_nonzero(x[:,40:61]) > count_nonzero(x[:,94:115]))
  - per frame t<200: features built from hand(sel by cond)/pose/lip coords,
    temporal diff vs frame t+1, static-pair distances, hand mask.

Sharding (8 cores, embarrassingly parallel over frames):
  - count phase: core c processes frames [12500c, 12500(c+1)). Hand elements
    are staged host-side as a dense transposed bf16 stream xs [128, 12500]
    (partitions 0-62 = lefth coords, 64-126 = righth, 63/127 zero pad) so
    the device streams contiguous bytes at full DMA rate. The nonzero
    reduction is split across engines: 6 chunks flow through PE
    dot-products with a +/-1 sign vector (indicators from a 4x-tier DVE
    not_equal, accumulated in PSUM), 4 chunks through a fused DVE
    accumulate chain whose last op also emits per-partition sums. All
    partials are exact small integers; the host combines them.
    (bf16 staging keeps nonzero-ness exactly for any |x| >= 2^-133; inputs
    are randn-distributed f32.)
  - feature phase: core c computes BOTH left/right feature variants for its
    output frames [25c, 25c+26) and writes yl_c/yr_c [25, 1198]. All
    feature inputs arrive in two packed DMAs (one bf16, one f32) to avoid
    descriptor-generation serialization; raw xfeat blocks are
    host-pre-assembled so on-device assembly is one copy + one
    temporal-diff subtract per variant; distances run as bf16 matmuls.
  - unshard: the host sums the partials, picks the variant
    (cond = cntL-cntR > 0), concatenates the per-core slices.
"""

import numpy as np
import ml_dtypes

import concourse.bass as bass
import concourse.tile as tile
from concourse import bacc, mybir
from concourse import bass_utils

F32 = mybir.dt.float32
BF16 = mybir.dt.bfloat16
ALU = mybir.AluOpType

NCORES = 8
T_TOT = 100000
SHARD = T_TOT // NCORES          # 12500 count frames per core
PC = 128                         # count partitions (63 lefth, pad, 63 righth, pad)
NCHUNK = 10                      # count chunks
CH = SHARD // NCHUNK             # 1250 frames per chunk
NPE = 6                          # chunks reduced via PE dot (0..NPE-1)
MMS = (512, 512, 226)            # moving-dim splits of a chunk (max 512)
OUTF = 25                        # output frames per core
BF = OUTF + 1                    # feature frames per core (1 halo)
KBW = 1091                       # bf16 pack width
KFW = 612                        # f32 pack width

# static pair index tables (match np.triu_indices order used by reference)
_HIU = np.triu_indices(21, 1)    # 210 hand pairs
_PIU = np.triu_indices(25, 1)    # 300 pose pairs
_LIU = np.triu_indices(20, 1)    # 190 lip pairs
NH, NP_, NL = 210, 300, 190


def _pairmat(nj, iu):
    g = np.zeros((nj, len(iu[0])), np.float32)
    g[iu[0], np.arange(len(iu[0]))] = 1.0
    g[iu[1], np.arange(len(iu[1]))] -= 1.0
    return g


def build_bass():
    nc = bacc.Bacc("TRN2", target_bir_lowering=False, debug=False,
                   num_devices=NCORES)

    xs = nc.dram_tensor("xs", [PC, SHARD], BF16, kind="ExternalInput")
    kb_d = nc.dram_tensor("kb", [PC, KBW], BF16, kind="ExternalInput")
    kf_d = nc.dram_tensor("kf", [BF, KFW], F32, kind="ExternalInput")
    yl = nc.dram_tensor("yl", [OUTF, 1198], F32, kind="ExternalOutput")
    yr = nc.dram_tensor("yr", [OUTF, 1198], F32, kind="ExternalOutput")
    pdr = nc.dram_tensor("pdr", [PC, 1], F32, kind="ExternalOutput")
    pdc = nc.dram_tensor("pdc", [1, CH], F32, kind="ExternalOutput")

    with tile.TileContext(nc) as tc:
        with (
            tc.tile_pool(name="cnt_in", bufs=NCHUNK) as cnt_in,
            tc.tile_pool(name="persist", bufs=1) as persist,
            tc.tile_pool(name="fb", bufs=1) as fb,
            tc.tile_pool(name="psum", bufs=4, space=bass.MemorySpace.PSUM) as psum,
            tc.tile_pool(name="psumc", bufs=1, space=bass.MemorySpace.PSUM) as psumc,
        ):
            # ---------------- packed feature inputs (2 DMAs, sync) ------
            KB = persist.tile([PC, KBW], BF16)
            nc.sync.dma_start(KB[:], kb_d[:])
            KF = persist.tile([BF, KFW], F32)
            nc.sync.dma_start(KF[:], kf_d[:])
            XR = KB[0:25, 0:390]
            gh = KB[0:21, 390:600]
            gp = KB[0:25, 600:900]
            gl = KB[0:20, 900:1090]
            sgn = KB[:, 1090:1091]
            XFR = KF[:, 0:153]
            XFL = KF[:, 153:306]
            XFRs = KF[0:OUTF, 306:459]
            XFLs = KF[0:OUTF, 459:612]

            # ---------------- count stream (gpsimd SWDGE) ---------------
            cts = []
            for k in range(NCHUNK):
                ts_ = cnt_in.tile([PC, CH], BF16, tag="cin")
                nc.gpsimd.dma_start(ts_[:], xs[:, k * CH:(k + 1) * CH])
                cts.append(ts_)

            acc = persist.tile([PC, CH], BF16)
            red = persist.tile([PC, 1], F32)
            pcs = [psumc.tile([1, m], F32, name=f"pc{i}", tag=f"pc{i}")
                   for i, m in enumerate(MMS)]

            def cnt_ne(k):
                nc.vector.tensor_scalar(
                    out=cts[k][:], in0=cts[k][:],
                    scalar1=0.0, scalar2=None, op0=ALU.not_equal)

            def cnt_mm(k):
                off = 0
                for i, m in enumerate(MMS):
                    nc.tensor.matmul(
                        pcs[i][:], sgn, cts[k][:, off:off + m],
                        start=(k == 0), stop=(k == NPE - 1),
                        skip_group_check=True)
                    off += m

            def cnt_chain(k):
                if k == NPE:
                    nc.vector.tensor_scalar(
                        out=acc[:], in0=cts[k][:],
                        scalar1=0.0, scalar2=None, op0=ALU.not_equal)
                else:
                    nc.vector.scalar_tensor_tensor(
                        out=acc[:], in0=cts[k][:], scalar=0.0, in1=acc[:],
                        op0=ALU.not_equal, op1=ALU.add,
                        accum_out=red[:] if k == NCHUNK - 1 else None)

            FEATL = fb.tile([OUTF, 1198], F32)
            FEATR = fb.tile([OUTF, 1198], F32)

            def v3(ft, lo, hi):
                return ft[:, lo:hi].rearrange("p (j c) -> p j c", c=3)

            def v2(ft, lo, hi):
                return ft[:, lo:hi].rearrange("p (j c) -> p j c", c=2)

            # ---- DVE stream (ordered by data arrival) ----
            cnt_ne(0)
            cnt_ne(1)
            # temporal diffs straight into the dxyz feature slices
            nc.vector.tensor_sub(FEATR[:, 153:306], XFR[0:OUTF, :], XFRs)
            nc.vector.tensor_sub(FEATL[:, 153:306], XFL[0:OUTF, :], XFLs)
            cnt_ne(2)
            # hand masks (sum over the selected hand's 63 coords)
            sumR = fb.tile([OUTF, 1], F32)
            nc.vector.reduce_sum(out=sumR[:], in_=XFR[0:OUTF, 0:63],
                                 axis=mybir.AxisListType.X)
            sumL = fb.tile([OUTF, 1], F32)
            nc.vector.reduce_sum(out=sumL[:], in_=XFL[0:OUTF, 0:63],
                                 axis=mybir.AxisListType.X)
            maskR = fb.tile([OUTF, 1], F32)
            nc.vector.tensor_scalar(out=maskR[:], in0=sumR[:],
                                    scalar1=0.0, scalar2=None,
                                    op0=ALU.not_equal)
            maskL = fb.tile([OUTF, 1], F32)
            nc.vector.tensor_scalar(out=maskL[:], in0=sumL[:],
                                    scalar1=0.0, scalar2=None,
                                    op0=ALU.not_equal)
            cnt_ne(3)
            # mirror x coords of the left variant (in place)
            for (lo, hi, cd) in ((153, 216, 3), (216, 306, 2)):
                vv = (v3 if cd == 3 else v2)(FEATL, lo, hi)
                nc.vector.tensor_scalar(
                    out=vv[:, :, 0:1], in0=vv[:, :, 0:1], scalar1=-1.0,
                    scalar2=None, op0=ALU.mult)
            cnt_ne(4)

            # ---- ACT stream ----
            nc.scalar.copy(FEATR[:, 0:153], XFR[0:OUTF, :])
            nc.scalar.copy(FEATL[:, 0:153], XFL[0:OUTF, :])

            # DVE flips of the copied raw blocks
            for (lo, hi, cd) in ((0, 63, 3), (63, 153, 2)):
                vv = (v3 if cd == 3 else v2)(FEATL, lo, hi)
                nc.vector.tensor_scalar(
                    out=vv[:, :, 0:1], in0=vv[:, :, 0:1], scalar1=-1.0,
                    scalar2=None, op0=ALU.mult)

            # ---- PE stream: all distance mms first, then count dots ----
            def dist2(dst, region, nj, gt, npair, ncoord):
                for c in range(ncoord):
                    pdsq = psum.tile([BF, npair], F32, tag="pdif")
                    base = region * 3 * BF + c * BF
                    nc.tensor.matmul(
                        pdsq[:], XR[0:nj, base:base + BF], gt)
                    if c == 0:
                        nc.scalar.square(dst[:], pdsq[:])
                    else:
                        sq = fb.tile([BF, npair], F32, tag="sqt")
                        nc.scalar.square(sq[:], pdsq[:])
                        nc.vector.tensor_add(dst[:], dst[:], sq[:])

            hd2L = fb.tile([BF, NH], F32)
            dist2(hd2L, 0, 21, gh, NH, 3)
            hd2R = fb.tile([BF, NH], F32)
            dist2(hd2R, 1, 21, gh, NH, 3)
            pd2 = fb.tile([BF, NP_], F32)
            dist2(pd2, 2, 25, gp, NP_, 2)
            ol2 = fb.tile([BF, NL], F32)
            dist2(ol2, 3, 20, gl, NL, 2)
            il2 = fb.tile([BF, NL], F32)
            dist2(il2, 4, 20, gl, NL, 2)

            cnt_ne(5)
            cnt_mm(0)
            cnt_mm(1)
            cnt_mm(2)
            cnt_mm(3)
            cnt_mm(4)
            cnt_mm(5)

            # ---- DVE chain for the tail chunks ----
            cnt_chain(6)
            cnt_chain(7)

            # ---- ACT: maskout columns, sqrts ----
            for FT, msk in ((FEATR, maskR), (FEATL, maskL)):
                nc.scalar.copy(FT[:, 1196:1197], msk[:])
                nc.scalar.add(FT[:, 1197:1198], msk[:], 1.0)

            nc.scalar.sqrt(FEATR[:, 306:516], hd2R[0:OUTF, :])
            nc.scalar.sqrt(FEATR[:, 516:816], pd2[0:OUTF, :])
            nc.scalar.sqrt(FEATR[:, 816:1006], ol2[0:OUTF, :])
            nc.scalar.sqrt(FEATR[:, 1006:1196], il2[0:OUTF, :])
            nc.sync.dma_start(yr[:], FEATR[:])

            nc.scalar.sqrt(FEATL[:, 306:516], hd2L[0:OUTF, :])
            # cond-invariant distance block: copy across
            nc.scalar.copy(FEATL[:, 516:1196], FEATR[:, 516:1196])
            nc.sync.dma_start(yl[:], FEATL[:])

            # PE-count PSUM rows -> SBUF -> DRAM (ACT, after its sqrts)
            pdrow = persist.tile([1, CH], F32)
            off = 0
            for i, m in enumerate(MMS):
                nc.scalar.copy(pdrow[:, off:off + m], pcs[i][:])
                off += m
            nc.sync.dma_start(pdc[:], pdrow[:])

            cnt_chain(8)
            cnt_chain(9)
            nc.sync.dma_start(pdr[:], red[:])

    nc.compile()
    return nc


_NC_CACHE = None


def _get_nc():
    global _NC_CACHE
    if _NC_CACHE is None:
        _NC_CACHE = build_bass()
    return _NC_CACHE


def make_in_maps(x: np.ndarray):
    x = np.ascontiguousarray(np.asarray(x, dtype=np.float32))
    assert x.shape == (T_TOT, 115, 3)
    xf = x.reshape(T_TOT, 345)
    # dense transposed bf16 hand stream: rows 0-62 lefth, 64-126 righth
    xlb = xf[:, 120:183].astype(ml_dtypes.bfloat16)   # [T,63]
    xrb = xf[:, 282:345].astype(ml_dtypes.bfloat16)   # [T,63]
    gh = _pairmat(21, _HIU)
    gp = _pairmat(25, _PIU)
    gl = _pairmat(20, _LIU)
    in_maps = []
    regions = ((40, 61), (94, 115), (61, 86), (0, 20), (20, 40))
    for c in range(NCORES):
        xs = np.zeros((PC, SHARD), ml_dtypes.bfloat16)
        xs[0:63] = xlb[c * SHARD:(c + 1) * SHARD].T
        xs[64:127] = xrb[c * SHARD:(c + 1) * SHARD].T
        xb = x[c * OUTF:c * OUTF + BF]                      # [26,115,3]
        xreg = np.zeros((25, 5 * 3 * BF), np.float32)
        for r, (j0, j1) in enumerate(regions):
            blk = xb[:, j0:j1, :].transpose(1, 2, 0)        # [J,3,BF]
            xreg[0:j1 - j0, r * 3 * BF:(r + 1) * 3 * BF] = \
                blk.reshape(j1 - j0, 3 * BF)
        # bf16 pack: xreg | gh | gp | gl | sgn
        kb = np.zeros((PC, KBW), ml_dtypes.bfloat16)
        kb[0:25, 0:390] = xreg.astype(ml_dtypes.bfloat16)
        kb[0:21, 390:600] = gh.astype(ml_dtypes.bfloat16)
        kb[0:25, 600:900] = gp.astype(ml_dtypes.bfloat16)
        kb[0:20, 900:1090] = gl.astype(ml_dtypes.bfloat16)
        kb[0:64, 1090] = 1.0
        kb[64:PC, 1090] = -1.0
        # f32 pack: xfr | xfl | xfrs | xfls  (hand63|pose xy 50|lip xy 40)
        def xfeat(hand_lo, hand_hi):
            return np.concatenate([
                xb[:, hand_lo:hand_hi, :].reshape(BF, 63),
                xb[:, 61:86, 0:2].reshape(BF, 50),
                xb[:, 0:20, 0:2].reshape(BF, 40)], axis=1)
        xfr = xfeat(94, 115)
        xfl = xfeat(40, 61)
        kf = np.zeros((BF, KFW), np.float32)
        kf[:, 0:153] = xfr
        kf[:, 153:306] = xfl
        kf[0:OUTF, 306:459] = xfr[1:BF]
        kf[0:OUTF, 459:612] = xfl[1:BF]
        in_maps.append({"xs": xs, "kb": kb, "kf": kf})
    return in_maps


def run_device(x: np.ndarray, **kw):
    nc = _get_nc()
    in_maps = make_in_maps(x)
    res = bass_utils.run_bass_kernel_spmd(
        nc, in_maps, core_ids=list(range(NCORES)), **kw)
    # global left/right decision from the exact integer-valued partials
    diff = 0.0
    for r in res.results:
        a = np.asarray(r["pdr"], dtype=np.float64)
        diff += a[0:64].sum() - a[64:128].sum()
        diff += np.asarray(r["pdc"], dtype=np.float64).sum()
    key = "yl" if diff > 0 else "yr"
    out = np.concatenate([r[key] for r in res.results], axis=0)
    return out.reshape(1, 200, 1198).astype(np.float32, copy=False), res


def kernel(x: np.ndarray) -> np.ndarray:
    return run_device(x)[0]


if __name__ == "__main__":
    rng = np.random.default_rng(0)
    x = rng.standard_normal((T_TOT, 115, 3), dtype=np.float32)
    out = kernel(x)
    print(out.shape, out.dtype, float(np.linalg.norm(out)))
